# revision 2
# baseline (speedup 1.0000x reference)
"""CharRNN (GRU, reset_after=True) Trainium2 kernel, v2.

Data parallel over batch (4096 -> 8 cores x 512). Per core, 512 batch =
2 pipelined chains x (4 quadrant-stacked groups x 64 cols). Group g of a
chain lives on partitions 32g:32g+20; every DVE/ACT instruction covers
all 4 groups at once (junk lanes between quadrant windows are kept
finite and never escape).

Per chain-step (one timestep of one chain):
  PE:  MMr/MMz accumulate U_r/U_z @ h onto pre-injected xz/xr PSUM
       (start=False); MMh computes hh = U_h h + br_h (ones row).
       xz/xr are injected by chunked identity matmuls (1 LDW amortized
       over 4 steps x 256 cols) -- keeps per-step K small and LDW count
       low.
  ACT: sigmoid(ps_r)->r_sb, sigmoid(ps_z)->z_sb (z off critical path),
       tanh(t2)->hc.
  DVE: t1 = r*hh, t2 = t1 + xh, v = hc*zn, h' = v + m1.
  GpSimd (off critical path): zn = 1 - z, m1 = z*h.
h' = z*h + (1-z)*hc computed as v+m1 so only 2 serial DVE ops follow
tanh on the chain.
"""

import os
import time

import numpy as np

import concourse.bacc as bacc
import concourse.tile as tile
from concourse import mybir
from concourse.bass_utils import run_bass_kernel_spmd

os.environ.setdefault("BASS_NEVER_TRACE", "1")

B, T, V, H, L = 4096, 256, 256, 20, 15
NCORES = 8
BC = B // NCORES          # 512 batch per core
NCH = 2                   # chains per core
G = 4                     # quadrant groups per chain
W = 64                    # batch columns per instruction
TCI = 4                   # steps per injection chunk
TCD = 16                  # steps per DMA chunk
NCD = T // TCD
KH = 117                  # hbuf rows: 4x(20 @ 32g) + ones @ 116
SPAN = 116                # elementwise partition span

_CACHE = {}


def _build_program():
    nc = bacc.Bacc("TRN2", target_bir_lowering=False, debug=False)
    f16 = mybir.dt.float16
    f32 = mybir.dt.float32
    AF = mybir.ActivationFunctionType

    # DRAM inputs (per core): per chain, per DMA chunk
    xzr = [
        nc.dram_tensor(f"xzr{c}", [NCD, 80, 2, TCD, W], f16, kind="ExternalInput")
        for c in range(NCH)
    ]
    xh = [
        nc.dram_tensor(f"xh{c}", [NCD, SPAN, TCD, W], f16, kind="ExternalInput")
        for c in range(NCH)
    ]
    wz = nc.dram_tensor("wz", [KH, SPAN], f16, kind="ExternalInput")
    wr = nc.dram_tensor("wr", [KH, SPAN], f16, kind="ExternalInput")
    wh = nc.dram_tensor("wh", [KH, SPAN], f16, kind="ExternalInput")
    isp = nc.dram_tensor("isp", [80, SPAN], f16, kind="ExternalInput")
    dwp = nc.dram_tensor("dwp", [KH, SPAN], f16, kind="ExternalInput")
    onesrow = nc.dram_tensor("onesrow", [1, W], f16, kind="ExternalInput")
    out = nc.dram_tensor("out", [NCH, SPAN, W], f32, kind="ExternalOutput")

    with tile.TileContext(nc) as tc:
        with (
            tc.tile_pool(name="consts", bufs=1) as consts,
            tc.tile_pool(name="xwpool", bufs=2) as xwpool,
            tc.tile_pool(name="work", bufs=2) as work,
            tc.tile_pool(name="pinj", bufs=2, space="PSUM") as pinj,
            tc.tile_pool(name="ph", bufs=2, space="PSUM") as ph,
        ):
            wz_sb = consts.tile([KH, SPAN], f16)
            wr_sb = consts.tile([KH, SPAN], f16)
            wh_sb = consts.tile([KH, SPAN], f16)
            isp_sb = consts.tile([80, SPAN], f16)
            dwp_sb = consts.tile([KH, SPAN], f16)
            nc.sync.dma_start(out=wz_sb, in_=wz.ap())
            nc.sync.dma_start(out=wr_sb, in_=wr.ap())
            nc.sync.dma_start(out=wh_sb, in_=wh.ap())
            nc.sync.dma_start(out=isp_sb, in_=isp.ap())
            nc.sync.dma_start(out=dwp_sb, in_=dwp.ap())

            ones_sb = consts.tile([SPAN, W], f16)
            nc.vector.memset(ones_sb, 1.0)

            hbuf = []
            for c in range(NCH):
                hb = consts.tile([KH, W], f16, name=f"hb{c}")
                nc.vector.memset(hb, 0.0)
                nc.sync.dma_start(out=hb[116:117, :], in_=onesrow.ap())
                hbuf.append(hb)

            def dma_chunk(k):
                ts = []
                for c in range(NCH):
                    xzr_sb = xwpool.tile([80, 2, TCD, W], f16, tag=f"xzr{c}")
                    xh_sb = xwpool.tile([SPAN, TCD, W], f16, tag=f"xh{c}")
                    nc.sync.dma_start(out=xzr_sb, in_=xzr[c].ap()[k])
                    nc.sync.dma_start(out=xh_sb, in_=xh[c].ap()[k])
                    ts.append((xzr_sb, xh_sb))
                return ts

            def inj_chunk(cur_dma, t):
                # inject xz/xr for steps [t, t+TCI) into fresh psum bufs
                k_in = (t // TCD, t % TCD)
                tiles = []
                for c in range(NCH):
                    pzr = pinj.tile(
                        [SPAN, 2 * TCI * W], f32, tag=f"pzr{c}", name=f"pzr{c}"
                    )
                    tiles.append(pzr)
                for c in range(NCH):
                    xzr_sb, _ = cur_dma[c]
                    pzr = tiles[c]
                    rhs = xzr_sb[:, :, k_in[1] : k_in[1] + TCI, :]
                    nc.tensor.matmul(
                        pzr, isp_sb, rhs, start=True,
                        stop=False, skip_group_check=True,
                    )
                return tiles

            cur_dma = dma_chunk(0)
            cur_inj = inj_chunk(cur_dma, 0)
            nxt_dma = None

            for t in range(T):
                kd, td = divmod(t, TCD)
                ti = t % TCI
                if td == 0 and kd + 1 < NCD:
                    nxt_dma = dma_chunk(kd + 1)

                # chain-major emission: all of chain c's step, then next chain
                for c in range(NCH):
                    pzr = cur_inj[c]
                    xh_sb = cur_dma[c][1]
                    psh = ph.tile([SPAN, W], f32, tag=f"ph{c}", name=f"psh{c}")
                    nc.tensor.matmul(
                        pzr[:, (TCI + ti) * W : (TCI + ti + 1) * W], wr_sb,
                        hbuf[c][:, :],
                        start=False, stop=True, skip_group_check=True,
                    )
                    nc.tensor.matmul(
                        psh, wh_sb, hbuf[c][:, :], start=True, stop=True
                    )
                    nc.tensor.matmul(
                        pzr[:, ti * W : (ti + 1) * W], wz_sb, hbuf[c][:, :],
                        start=False, stop=True, skip_group_check=True,
                    )
                    r_sb = work.tile([SPAN, W], f16, tag=f"r{c}", name=f"r{c}")
                    nc.scalar.activation(
                        r_sb, pzr[:, (TCI + ti) * W : (TCI + ti + 1) * W],
                        AF.Sigmoid,
                    )
                    t1_sb = work.tile([SPAN, W], f16, tag=f"t1{c}", name=f"t1{c}")
                    nc.vector.tensor_mul(t1_sb, r_sb, psh)
                    t2_sb = work.tile([SPAN, W], f16, tag=f"t2{c}", name=f"t2{c}")
                    nc.vector.tensor_add(t2_sb, t1_sb, xh_sb[:, td, :])
                    hc_sb = work.tile([SPAN, W], f16, tag=f"hc{c}", name=f"hc{c}")
                    nc.scalar.activation(hc_sb, t2_sb, AF.Tanh)
                    # sigma_z emitted after tanh: keeps tanh off the ACT
                    # queue's back; z is only needed at the m multiply
                    z_sb = work.tile([SPAN, W], f16, tag=f"z{c}", name=f"z{c}")
                    nc.scalar.activation(
                        z_sb, pzr[:, ti * W : (ti + 1) * W], AF.Sigmoid
                    )
                    # h' = hc + z*(h - hc): rounds only the small correction
                    d_sb = work.tile([SPAN, W], f16, tag=f"d{c}", name=f"d{c}")
                    nc.vector.tensor_sub(d_sb, hbuf[c][0:SPAN, :], hc_sb)
                    m_sb = work.tile([SPAN, W], f16, tag=f"m{c}", name=f"m{c}")
                    nc.vector.tensor_mul(m_sb, z_sb, d_sb)
                    nc.vector.tensor_add(
                        hbuf[c][0:SPAN, :], hc_sb, m_sb
                    )

                # next injection chunk (consumes DMA tiles of its steps)
                if ti == TCI - 1 and t + 1 < T:
                    use_dma = nxt_dma if (t + 1) % TCD == 0 else cur_dma
                    cur_inj = inj_chunk(use_dma, t + 1)
                if td == TCD - 1 and nxt_dma is not None:
                    cur_dma = nxt_dma
                    nxt_dma = None

            # dense epilogue
            for c in range(NCH):
                po = ph.tile([SPAN, W], f32, tag=f"ph{c}", name=f"po{c}")
                nc.tensor.matmul(po, dwp_sb, hbuf[c][:, :], start=True, stop=True)
                o_sb = work.tile([SPAN, W], f32, tag=f"o{c}")
                nc.scalar.activation(o_sb, po, AF.Identity)
                nc.sync.dma_start(out=out.ap()[c], in_=o_sb)

    nc.compile()
    return nc


def _get_program():
    if "nc" not in _CACHE:
        _CACHE["nc"] = _build_program()
    return _CACHE["nc"]


def _prepare_inputs(x, kernel, recurrent_kernel, bias, dense_w, dense_b):
    x = np.asarray(x)
    kernel = np.asarray(kernel, dtype=np.float32)
    rk = np.asarray(recurrent_kernel, dtype=np.float32)
    bias = np.asarray(bias, dtype=np.float32)
    dense_w = np.asarray(dense_w, dtype=np.float32)
    dense_b = np.asarray(dense_b, dtype=np.float32)
    f16 = np.float16

    # gate tables: ktab = kernel + input_bias; fold recurrent z/r bias too
    ktab = kernel + bias[0]
    ktab[:, 0 : 2 * H] += bias[1][0 : 2 * H]
    ktab = ktab.astype(f16)
    uz = rk[:, 0:H]
    ur = rk[:, H : 2 * H]
    uh = rk[:, 2 * H : 3 * H]
    br_h = bias[1][2 * H : 3 * H]

    def umap(u, with_bias=None):
        w = np.zeros((KH, SPAN), np.float32)
        for g in range(G):
            w[32 * g : 32 * g + H, 32 * g : 32 * g + H] = u
            if with_bias is not None:
                w[116, 32 * g : 32 * g + H] = with_bias
        return w.astype(f16)

    isp_np = np.zeros((80, SPAN), np.float32)
    for g in range(G):
        for i in range(H):
            isp_np[20 * g + i, 32 * g + i] = 1.0

    dwp_np = np.zeros((KH, SPAN), np.float32)
    for g in range(G):
        dwp_np[32 * g : 32 * g + H, 32 * g : 32 * g + L] = dense_w
        dwp_np[116, 32 * g : 32 * g + L] = dense_b

    common = {
        "wz": umap(uz),
        "wr": umap(ur),
        "wh": umap(uh, with_bias=br_h),
        "isp": isp_np.astype(f16),
        "dwp": dwp_np.astype(f16),
        "onesrow": np.ones((1, W), f16),
    }

    in_maps = []
    for core in range(NCORES):
        xc = x[core * BC : (core + 1) * BC]        # [BC, T]
        xw = ktab[xc]                              # [BC, T, 60] f16
        mm = dict(common)
        for c in range(NCH):
            # chain c: batch c*256 + g*64 + j
            xwc = xw[c * 256 : (c + 1) * 256]      # [256, T, 60]
            # [G, W, T, 60] -> gate blocks
            xwc = xwc.reshape(G, W, T, 3 * H)
            # packed [80, 2, T, W] layout for xz|xr
            xzr_np = np.empty((80, 2, NCD, TCD, W), f16)
            xh_np = np.zeros((SPAN, NCD, TCD, W), f16)
            for g in range(G):
                blk = xwc[g]                       # [W, T, 60]
                tz = blk[:, :, 0:H].transpose(2, 1, 0).reshape(H, NCD, TCD, W)
                tr = blk[:, :, H : 2 * H].transpose(2, 1, 0).reshape(
                    H, NCD, TCD, W
                )
                th = blk[:, :, 2 * H : 3 * H].transpose(2, 1, 0).reshape(
                    H, NCD, TCD, W
                )
                xzr_np[20 * g : 20 * g + H, 0] = tz
                xzr_np[20 * g : 20 * g + H, 1] = tr
                xh_np[32 * g : 32 * g + H] = th
            mm[f"xzr{c}"] = np.ascontiguousarray(xzr_np.transpose(2, 0, 1, 3, 4))
            mm[f"xh{c}"] = np.ascontiguousarray(xh_np.transpose(1, 0, 2, 3))
        in_maps.append(mm)
    return in_maps


def run(inputs, trace=False):
    nc = _get_program()
    in_maps = _prepare_inputs(
        inputs["x"],
        inputs["kernel"],
        inputs["recurrent_kernel"],
        inputs["bias"],
        inputs["dense_w"],
        inputs["dense_b"],
    )
    res = None
    last_err = None
    for attempt in range(4):
        try:
            res = run_bass_kernel_spmd(
                nc, in_maps, core_ids=list(range(NCORES)), trace=trace
            )
            break
        except Exception as e:
            last_err = e
            try:
                import jax

                jax.clear_caches()
                import jax.extend.backend as _jeb

                _jeb.clear_backends()
            except Exception:
                pass
            time.sleep(3.0)
    if res is None:
        raise last_err
    logits = np.empty((B, L), dtype=np.float32)
    for core in range(NCORES):
        o = res.results[core]["out"]              # [NCH, SPAN, W]
        for c in range(NCH):
            for g in range(G):
                blk = o[c, 32 * g : 32 * g + L, :]  # [L, W]
                b0 = core * BC + c * 256 + g * W
                logits[b0 : b0 + W] = blk.T
    return logits, res.exec_time_ns


def kernel(**inputs) -> np.ndarray:
    logits, _ = run(inputs, trace=False)
    return logits


# revision 3
# speedup vs baseline: 1.0010x; 1.0010x over previous
"""CharRNN (GRU, reset_after=True) Trainium2 kernel, v2.

Data parallel over batch (4096 -> 8 cores x 512). Per core, 512 batch =
2 pipelined chains x (4 quadrant-stacked groups x 64 cols). Group g of a
chain lives on partitions 32g:32g+20; every DVE/ACT instruction covers
all 4 groups at once (junk lanes between quadrant windows are kept
finite and never escape).

Per chain-step (one timestep of one chain):
  PE:  MMr/MMz accumulate U_r/U_z @ h onto pre-injected xz/xr PSUM
       (start=False); MMh computes hh = U_h h + br_h (ones row).
       xz/xr are injected by chunked identity matmuls (1 LDW amortized
       over 4 steps x 256 cols) -- keeps per-step K small and LDW count
       low.
  ACT: sigmoid(ps_r)->r_sb, sigmoid(ps_z)->z_sb (z off critical path),
       tanh(t2)->hc.
  DVE: t1 = r*hh, t2 = t1 + xh, v = hc*zn, h' = v + m1.
  GpSimd (off critical path): zn = 1 - z, m1 = z*h.
h' = z*h + (1-z)*hc computed as v+m1 so only 2 serial DVE ops follow
tanh on the chain.
"""

import os
import time

import numpy as np

import concourse.bacc as bacc
import concourse.tile as tile
from concourse import mybir
from concourse.bass_utils import run_bass_kernel_spmd

os.environ.setdefault("BASS_NEVER_TRACE", "1")

B, T, V, H, L = 4096, 256, 256, 20, 15
NCORES = 8
BC = B // NCORES          # 512 batch per core
NCH = 2                   # chains per core
G = 4                     # quadrant groups per chain
W = 64                    # batch columns per instruction
TCI = 4                   # steps per injection chunk
TCD = 16                  # steps per DMA chunk
NCD = T // TCD
KH = 117                  # hbuf rows: 4x(20 @ 32g) + ones @ 116
SPAN = 116                # elementwise partition span

_CACHE = {}


def _build_program():
    nc = bacc.Bacc("TRN2", target_bir_lowering=False, debug=False)
    f16 = mybir.dt.float16
    f32 = mybir.dt.float32
    AF = mybir.ActivationFunctionType

    # DRAM inputs (per core): per chain, per DMA chunk
    xzr = [
        nc.dram_tensor(f"xzr{c}", [NCD, 80, 2, TCD, W], f16, kind="ExternalInput")
        for c in range(NCH)
    ]
    xh = [
        nc.dram_tensor(f"xh{c}", [NCD, SPAN, TCD, W], f16, kind="ExternalInput")
        for c in range(NCH)
    ]
    wz = nc.dram_tensor("wz", [KH, SPAN], f16, kind="ExternalInput")
    wrn = nc.dram_tensor("wrn", [SPAN, SPAN], f16, kind="ExternalInput")
    wh = nc.dram_tensor("wh", [KH, SPAN], f16, kind="ExternalInput")
    isp = nc.dram_tensor("isp", [80, SPAN], f16, kind="ExternalInput")
    dwp = nc.dram_tensor("dwp", [KH, SPAN], f16, kind="ExternalInput")
    onesrow = nc.dram_tensor("onesrow", [1, W], f16, kind="ExternalInput")
    out = nc.dram_tensor("out", [NCH, SPAN, W], f32, kind="ExternalOutput")

    with tile.TileContext(nc) as tc:
        with (
            tc.tile_pool(name="consts", bufs=1) as consts,
            tc.tile_pool(name="xwpool", bufs=2) as xwpool,
            tc.tile_pool(name="work", bufs=2) as work,
            tc.tile_pool(name="pinj", bufs=2, space="PSUM") as pinj,
            tc.tile_pool(name="ph", bufs=2, space="PSUM") as ph,
        ):
            wz_sb = consts.tile([KH, SPAN], f16)
            wrn_sb = consts.tile([SPAN, SPAN], f16)
            wh_sb = consts.tile([KH, SPAN], f16)
            isp_sb = consts.tile([80, SPAN], f16)
            dwp_sb = consts.tile([KH, SPAN], f16)
            nc.sync.dma_start(out=wz_sb, in_=wz.ap())
            nc.sync.dma_start(out=wrn_sb, in_=wrn.ap())
            nc.sync.dma_start(out=wh_sb, in_=wh.ap())
            nc.sync.dma_start(out=isp_sb, in_=isp.ap())
            nc.sync.dma_start(out=dwp_sb, in_=dwp.ap())

            ones_sb = consts.tile([SPAN, W], f16)
            nc.vector.memset(ones_sb, 1.0)

            hbuf = []
            for c in range(NCH):
                hb = consts.tile([KH, W], f16, name=f"hb{c}")
                nc.vector.memset(hb, 0.0)
                nc.sync.dma_start(out=hb[116:117, :], in_=onesrow.ap())
                hbuf.append(hb)
            zeros_sb = consts.tile([SPAN, W], f16)
            nc.vector.memset(zeros_sb, 0.0)
            hc_prev = [zeros_sb] * NCH
            m_prev = [zeros_sb] * NCH

            def dma_chunk(k):
                ts = []
                for c in range(NCH):
                    xzr_sb = xwpool.tile([80, 2, TCD, W], f16, tag=f"xzr{c}")
                    xh_sb = xwpool.tile([SPAN, TCD, W], f16, tag=f"xh{c}")
                    nc.sync.dma_start(out=xzr_sb, in_=xzr[c].ap()[k])
                    nc.sync.dma_start(out=xh_sb, in_=xh[c].ap()[k])
                    ts.append((xzr_sb, xh_sb))
                return ts

            def inj_chunk(cur_dma, t):
                # inject xz/xr for steps [t, t+TCI) into fresh psum bufs
                k_in = (t // TCD, t % TCD)
                tiles = []
                for c in range(NCH):
                    pzr = pinj.tile(
                        [SPAN, 2 * TCI * W], f32, tag=f"pzr{c}", name=f"pzr{c}"
                    )
                    tiles.append(pzr)
                for c in range(NCH):
                    xzr_sb, _ = cur_dma[c]
                    pzr = tiles[c]
                    rhs = xzr_sb[:, :, k_in[1] : k_in[1] + TCI, :]
                    nc.tensor.matmul(
                        pzr, isp_sb, rhs, start=True,
                        stop=False, skip_group_check=True,
                    )
                return tiles

            cur_dma = dma_chunk(0)
            cur_inj = inj_chunk(cur_dma, 0)
            nxt_dma = None

            for t in range(T):
                kd, td = divmod(t, TCD)
                ti = t % TCI
                if td == 0 and kd + 1 < NCD:
                    nxt_dma = dma_chunk(kd + 1)

                # chain-major emission: all of chain c's step, then next chain
                for c in range(NCH):
                    pzr = cur_inj[c]
                    xh_sb = cur_dma[c][1]
                    psh = ph.tile([SPAN, W], f32, tag=f"ph{c}", name=f"psh{c}")
                    # r-gate split: U_r@hc(t-1) pre-executes during the
                    # tail (hc ready at tanh); U_r@m(t-1) accumulates last,
                    # so the chain entry is m -> MMr_m -> sigmoid_r and the
                    # h'-add drops off the critical path (h' only feeds
                    # MMh/MMz, which have ~1.3us of slack).
                    nc.tensor.matmul(
                        pzr[:, (TCI + ti) * W : (TCI + ti + 1) * W], wrn_sb,
                        hc_prev[c][0:SPAN, :],
                        start=False, stop=False, skip_group_check=True,
                    )
                    nc.tensor.matmul(
                        pzr[:, (TCI + ti) * W : (TCI + ti + 1) * W], wrn_sb,
                        m_prev[c][0:SPAN, :],
                        start=False, stop=True, skip_group_check=True,
                    )
                    nc.tensor.matmul(
                        psh, wh_sb, hbuf[c][:, :], start=True, stop=True
                    )
                    nc.tensor.matmul(
                        pzr[:, ti * W : (ti + 1) * W], wz_sb, hbuf[c][:, :],
                        start=False, stop=True, skip_group_check=True,
                    )
                    r_sb = work.tile([SPAN, W], f16, tag=f"r{c}", name=f"r{c}")
                    nc.scalar.activation(
                        r_sb, pzr[:, (TCI + ti) * W : (TCI + ti + 1) * W],
                        AF.Sigmoid,
                    )
                    t1_sb = work.tile([SPAN, W], f16, tag=f"t1{c}", name=f"t1{c}")
                    nc.vector.tensor_mul(t1_sb, r_sb, psh)
                    t2_sb = work.tile([SPAN, W], f16, tag=f"t2{c}", name=f"t2{c}")
                    nc.vector.tensor_add(t2_sb, t1_sb, xh_sb[:, td, :])
                    hc_sb = work.tile([SPAN, W], f16, tag=f"hc{c}", name=f"hc{c}")
                    nc.scalar.activation(hc_sb, t2_sb, AF.Tanh)
                    # sigma_z emitted after tanh: keeps tanh off the ACT
                    # queue's back; z is only needed at the m multiply
                    z_sb = work.tile([SPAN, W], f16, tag=f"z{c}", name=f"z{c}")
                    nc.scalar.activation(
                        z_sb, pzr[:, ti * W : (ti + 1) * W], AF.Sigmoid
                    )
                    # h' = hc + z*(h - hc): rounds only the small correction
                    d_sb = work.tile([SPAN, W], f16, tag=f"d{c}", name=f"d{c}")
                    nc.vector.tensor_sub(d_sb, hbuf[c][0:SPAN, :], hc_sb)
                    m_sb = work.tile([SPAN, W], f16, tag=f"m{c}", name=f"m{c}")
                    nc.vector.tensor_mul(m_sb, z_sb, d_sb)
                    nc.vector.tensor_add(
                        hbuf[c][0:SPAN, :], hc_sb, m_sb
                    )
                    hc_prev[c] = hc_sb
                    m_prev[c] = m_sb

                # next injection chunk (consumes DMA tiles of its steps)
                if ti == TCI - 1 and t + 1 < T:
                    use_dma = nxt_dma if (t + 1) % TCD == 0 else cur_dma
                    cur_inj = inj_chunk(use_dma, t + 1)
                if td == TCD - 1 and nxt_dma is not None:
                    cur_dma = nxt_dma
                    nxt_dma = None

            # dense epilogue
            for c in range(NCH):
                po = ph.tile([SPAN, W], f32, tag=f"ph{c}", name=f"po{c}")
                nc.tensor.matmul(po, dwp_sb, hbuf[c][:, :], start=True, stop=True)
                o_sb = work.tile([SPAN, W], f32, tag=f"o{c}")
                nc.scalar.activation(o_sb, po, AF.Identity)
                nc.sync.dma_start(out=out.ap()[c], in_=o_sb)

    nc.compile()
    return nc


def _get_program():
    if "nc" not in _CACHE:
        _CACHE["nc"] = _build_program()
    return _CACHE["nc"]


def _prepare_inputs(x, kernel, recurrent_kernel, bias, dense_w, dense_b):
    x = np.asarray(x)
    kernel = np.asarray(kernel, dtype=np.float32)
    rk = np.asarray(recurrent_kernel, dtype=np.float32)
    bias = np.asarray(bias, dtype=np.float32)
    dense_w = np.asarray(dense_w, dtype=np.float32)
    dense_b = np.asarray(dense_b, dtype=np.float32)
    f16 = np.float16

    # gate tables: ktab = kernel + input_bias; fold recurrent z/r bias too
    ktab = kernel + bias[0]
    ktab[:, 0 : 2 * H] += bias[1][0 : 2 * H]
    ktab = ktab.astype(f16)
    uz = rk[:, 0:H]
    ur = rk[:, H : 2 * H]
    uh = rk[:, 2 * H : 3 * H]
    br_h = bias[1][2 * H : 3 * H]

    def umap(u, with_bias=None):
        w = np.zeros((KH, SPAN), np.float32)
        for g in range(G):
            w[32 * g : 32 * g + H, 32 * g : 32 * g + H] = u
            if with_bias is not None:
                w[116, 32 * g : 32 * g + H] = with_bias
        return w.astype(f16)

    isp_np = np.zeros((80, SPAN), np.float32)
    for g in range(G):
        for i in range(H):
            isp_np[20 * g + i, 32 * g + i] = 1.0

    dwp_np = np.zeros((KH, SPAN), np.float32)
    for g in range(G):
        dwp_np[32 * g : 32 * g + H, 32 * g : 32 * g + L] = dense_w
        dwp_np[116, 32 * g : 32 * g + L] = dense_b

    common = {
        "wz": umap(uz),
        "wrn": umap(ur)[0:SPAN],
        "wh": umap(uh, with_bias=br_h),
        "isp": isp_np.astype(f16),
        "dwp": dwp_np.astype(f16),
        "onesrow": np.ones((1, W), f16),
    }

    in_maps = []
    for core in range(NCORES):
        xc = x[core * BC : (core + 1) * BC]        # [BC, T]
        xw = ktab[xc]                              # [BC, T, 60] f16
        mm = dict(common)
        for c in range(NCH):
            # chain c: batch c*256 + g*64 + j
            xwc = xw[c * 256 : (c + 1) * 256]      # [256, T, 60]
            # [G, W, T, 60] -> gate blocks
            xwc = xwc.reshape(G, W, T, 3 * H)
            # packed [80, 2, T, W] layout for xz|xr
            xzr_np = np.empty((80, 2, NCD, TCD, W), f16)
            xh_np = np.zeros((SPAN, NCD, TCD, W), f16)
            for g in range(G):
                blk = xwc[g]                       # [W, T, 60]
                tz = blk[:, :, 0:H].transpose(2, 1, 0).reshape(H, NCD, TCD, W)
                tr = blk[:, :, H : 2 * H].transpose(2, 1, 0).reshape(
                    H, NCD, TCD, W
                )
                th = blk[:, :, 2 * H : 3 * H].transpose(2, 1, 0).reshape(
                    H, NCD, TCD, W
                )
                xzr_np[20 * g : 20 * g + H, 0] = tz
                xzr_np[20 * g : 20 * g + H, 1] = tr
                xh_np[32 * g : 32 * g + H] = th
            mm[f"xzr{c}"] = np.ascontiguousarray(xzr_np.transpose(2, 0, 1, 3, 4))
            mm[f"xh{c}"] = np.ascontiguousarray(xh_np.transpose(1, 0, 2, 3))
        in_maps.append(mm)
    return in_maps


def run(inputs, trace=False):
    nc = _get_program()
    in_maps = _prepare_inputs(
        inputs["x"],
        inputs["kernel"],
        inputs["recurrent_kernel"],
        inputs["bias"],
        inputs["dense_w"],
        inputs["dense_b"],
    )
    res = None
    last_err = None
    for attempt in range(4):
        try:
            res = run_bass_kernel_spmd(
                nc, in_maps, core_ids=list(range(NCORES)), trace=trace
            )
            break
        except Exception as e:
            last_err = e
            try:
                import jax

                jax.clear_caches()
                import jax.extend.backend as _jeb

                _jeb.clear_backends()
            except Exception:
                pass
            time.sleep(3.0)
    if res is None:
        raise last_err
    logits = np.empty((B, L), dtype=np.float32)
    for core in range(NCORES):
        o = res.results[core]["out"]              # [NCH, SPAN, W]
        for c in range(NCH):
            for g in range(G):
                blk = o[c, 32 * g : 32 * g + L, :]  # [L, W]
                b0 = core * BC + c * 256 + g * W
                logits[b0 : b0 + W] = blk.T
    return logits, res.exec_time_ns


def kernel(**inputs) -> np.ndarray:
    logits, _ = run(inputs, trace=False)
    return logits
